# revision 3
# baseline (speedup 1.0000x reference)
"""CORLoss Trainium2 kernel.

Reference (per row of N=128):
    mean1 = mean(d1) + EPS ; mean2 = mean(d2) + EPS
    std1, std2 unbiased ; cov = sum((d1-mean1)*(d2-mean2))/(n-1)
    cor  = (cov / (std1*std2 + EPS)) ** 3
    tl1  = -log((cor + 1 + EPS)/2)
    tl2  = mean(|softmax(d1) - softmax(d2)|)
    a = |cor| ; loss_row = a*tl1 + (1-a)*tl2
    out  = sum(loss_row) over all B rows, shape (1,)

Strategy: data-parallel over 8 NeuronCores, 16384 rows/core, streamed as
[128 partitions, 16 blocks, 128] supertiles (one row per (partition,
block)).  Per-row sums are all free-dim reductions:

  s1,s2   = sum(d)            DVE fp32 segmented reduce
  q1,q2   = sum(d^2)          ACT Square -> bf16, DVE fold+reduce
  s12     = sum(d1*d2)        GP multiply -> bf16, DVE fold+reduce
  se1,se2 = sum(exp(d))       ACT Exp -> bf16, DVE fold+reduce
  T       = sum|e1-c*e2|      GP mult/sub (c=se1/se2 broadcast), ACT Abs,
                              DVE fold+reduce

bf16 intermediates let the DVE "fold" adds (tensor_tensor bf16 = 2
elem/cycle) halve reduce element counts; all reductions accumulate the
final sums in fp32.  exp is computed without max-subtraction (inputs are
standard normal, fp32-safe).  Input loads alternate between the SP and ACT
HWDGE rings so the two 8MB streams transfer concurrently.  A vectorized
epilogue turns the eight per-row statistics into per-row losses and one
[128,1] partial sum per core; the host adds the 8*128 partials.
"""

import sys

sys.path.insert(0, "/opt/trn_rl_repo")

import numpy as np

import concourse.bass as bass
import concourse.tile as tile
from concourse import mybir
from concourse.vector_clock import ScopedClock

B, N = 131072, 128
EPS = 1e-3
N_CORES = 8
R = B // N_CORES          # rows per core = 16384
ST_ROWS = 2048            # rows per supertile
NB = ST_ROWS // 128       # 16 row-blocks per supertile
NST = R // ST_ROWS        # 8 supertiles per core
NCOLS = R // 128          # 128 stat columns per core
F32 = mybir.dt.float32
BF16 = mybir.dt.bfloat16
Alu = mybir.AluOpType
Act = mybir.ActivationFunctionType


def _tt(nc, out, a, b, op):
    nc.vector.tensor_tensor(out=out, in0=a, in1=b, op=op)


def split_waits(nc, cap=1):
    """This walrus build rejects instructions carrying more than ~1 inline
    semaphore wait; move excess waits onto fresh same-engine nops placed
    immediately before the instruction."""
    for fn in nc.m.functions:
        for bb in fn.blocks:
            snapshot = list(bb.instructions)
            out = []
            for inst in snapshot:
                si = inst.sync_info
                if si is not None and si.on_wait and len(si.on_wait) > cap:
                    waits = list(si.on_wait)
                    extra, keep = waits[:-cap], waits[-cap:]
                    while si.on_wait:
                        si.on_wait.pop()
                    for w in keep:
                        si.on_wait.append(w)
                    for w in extra:
                        bi = nc.engines[inst.engine].nop(nofuse=True, hint="wsplit")
                        nop_inst = bi.ins
                        for fb in nc.m.functions[0].blocks:
                            if fb.instructions and fb.instructions[-1] is nop_inst:
                                fb.instructions.pop()
                                break
                        nop_inst.sync_info = mybir.SyncInfo(on_wait=[w], on_update=[])
                        out.append(nop_inst)
                out.append(inst)
            bb.instructions[:] = out


def build_body(
    nc, tc, d1, d2, y,
    data_pool, exp_pool, g_pool, fold_pool, small_pool, stats_pool, epi_pool,
):
    if True:
        if True:
            def starr(name):
                return stats_pool.tile([128, NCOLS], F32, tag=name, name=name)

            s1a, s2a = starr("s1a"), starr("s2a")
            # five fused stat rows: 0=q1 1=q2 2=s12 3=se1 4=se2
            statsA = stats_pool.tile([128, 5, NCOLS], F32, tag="statsA", name="statsA")
            ta = stats_pool.tile([128, 1, NCOLS], F32, tag="ta", name="ta")

            def fold_reduce(src_bf16, out_col, chain, nch, splits=None):
                """src [128,nch,NB,128] bf16 -> out_col [128,nch,NB] f32 via
                two bf16 halving adds (DVE 2x mode) + segmented reduces.
                `splits` partitions the chain dim so urgent stats (se1/se2)
                become available before the rest."""
                h1 = fold_pool.tile(
                    [128, nch, NB, 64], BF16, tag=f"h1{chain}", name=f"h1{chain}"
                )
                _tt(nc, h1, src_bf16[:, :, :, 0:64], src_bf16[:, :, :, 64:128], Alu.add)
                h2 = fold_pool.tile(
                    [128, nch, NB, 32], BF16, tag=f"h2{chain}", name=f"h2{chain}"
                )
                _tt(nc, h2, h1[:, :, :, 0:32], h1[:, :, :, 32:64], Alu.add)
                h3 = fold_pool.tile(
                    [128, nch, NB, 16], BF16, tag=f"h3{chain}", name=f"h3{chain}"
                )
                _tt(nc, h3, h2[:, :, :, 0:16], h2[:, :, :, 16:32], Alu.add)
                for lo, hi in splits or [(0, nch)]:
                    nc.vector.reduce_sum(
                        out=out_col[:, lo:hi], in_=h3[:, lo:hi],
                        axis=mybir.AxisListType.X,
                    )

            for st in range(NST):
                rows = slice(st * ST_ROWS, (st + 1) * ST_ROWS)
                cols = slice(st * NB, (st + 1) * NB)
                # one row per (partition, block); 16 consecutive rows per
                # partition -> 8KB contiguous DMA per partition
                src1 = d1[rows, :].rearrange("(p b) n -> p b n", p=128)
                src2 = d2[rows, :].rearrange("(p b) n -> p b n", p=128)

                t1 = data_pool.tile([128, NB, N], F32, tag="t1", name="t1")
                t2 = data_pool.tile([128, NB, N], F32, tag="t2", name="t2")
                # both on the SP HWDGE ring: the ACT ring would charge the
                # scalar engine, which is busier than the SP queue
                nc.sync.dma_start(out=t1, in_=src1)
                nc.sync.dma_start(out=t2, in_=src2)

                # plain sums (fp32)
                nc.vector.reduce_sum(
                    out=s1a[:, cols], in_=t1, axis=mybir.AxisListType.X
                )
                nc.vector.reduce_sum(
                    out=s2a[:, cols], in_=t2, axis=mybir.AxisListType.X
                )

                # five bf16 derived streams packed into one wide tile so the
                # fold/reduce instructions amortize: 0=sq1 1=sq2 2=prod 3=e1 4=e2
                bigA = exp_pool.tile([128, 5, NB, N], BF16, tag="bigA", name="bigA")
                nc.scalar.activation(out=bigA[:, 0], in_=t1, func=Act.Square)
                nc.scalar.activation(out=bigA[:, 1], in_=t2, func=Act.Square)
                nc.gpsimd.tensor_tensor(out=bigA[:, 2], in0=t1, in1=t2, op=Alu.mult)
                nc.scalar.activation(out=bigA[:, 3], in_=t1, func=Act.Exp)
                nc.scalar.activation(out=bigA[:, 4], in_=t2, func=Act.Exp)
                fold_reduce(
                    bigA, statsA[:, :, cols], "A", 5, splits=[(3, 5), (0, 3)]
                )

                # T = sum |e1 - (se1/se2)*e2| per row:
                # g = (e2*c) - e1 per block on DVE (bf16 stt, 2x mode),
                # |g| on ACT, fold on GPSIMD, reduce on DVE
                e1 = bigA[:, 3]
                e2 = bigA[:, 4]
                rc = small_pool.tile([128, NB], F32, tag="rc", name="rc")
                c = small_pool.tile([128, NB], F32, tag="c", name="c")
                nc.vector.reciprocal(out=rc, in_=statsA[:, 4, cols])
                _tt(nc, c, statsA[:, 3, cols], rc, Alu.mult)
                cb = c.broadcast_to([128, NB, N])
                f = g_pool.tile([128, NB, N], BF16, tag="f", name="f")
                nc.gpsimd.tensor_tensor(out=f, in0=e2, in1=cb, op=Alu.mult)
                g = g_pool.tile([128, NB, N], BF16, tag="g", name="g")
                _tt(nc, g, e1, f, Alu.subtract)
                ag = g_pool.tile([128, 1, NB, N], BF16, tag="ag", name="ag")
                nc.scalar.activation(out=ag[:, 0], in_=g, func=Act.Abs)
                fold_reduce(ag, ta[:, :, cols], "B", 1)

            # ---- per-row epilogue on [128, NCOLS] stat tiles ----
            def ep(name):
                return epi_pool.tile([128, NCOLS], F32, tag=name, name=name)

            # M2 = q - s^2/n ; num = s12 - s1*s2/n + n*EPS^2
            q1a, q2a = statsA[:, 0, :], statsA[:, 1, :]
            s12a = statsA[:, 2, :]
            se1a = statsA[:, 3, :]
            u1, m2_1 = ep("u1"), ep("m2_1")
            _tt(nc, u1, s1a, s1a, Alu.mult)
            nc.vector.scalar_tensor_tensor(
                out=m2_1, in0=u1, scalar=-1.0 / N, in1=q1a, op0=Alu.mult, op1=Alu.add
            )
            u2, m2_2 = ep("u2"), ep("m2_2")
            _tt(nc, u2, s2a, s2a, Alu.mult)
            nc.vector.scalar_tensor_tensor(
                out=m2_2, in0=u2, scalar=-1.0 / N, in1=q2a, op0=Alu.mult, op1=Alu.add
            )
            u, num, w = ep("u"), ep("num"), ep("w")
            _tt(nc, u, s1a, s2a, Alu.mult)
            nc.vector.scalar_tensor_tensor(
                out=num, in0=u, scalar=-1.0 / N, in1=s12a, op0=Alu.mult, op1=Alu.add
            )
            _tt(nc, w, m2_1, m2_2, Alu.mult)

            # cor = (num + n*EPS^2) / (sqrt(w) + (n-1)*EPS), one Newton step
            # on the low-precision ACT sqrt
            sp, rsp, spn = ep("sp"), ep("rsp"), ep("spn")
            nc.scalar.activation(out=sp, in_=w, func=Act.Sqrt)
            nc.vector.reciprocal(out=rsp, in_=sp)
            _tt(nc, rsp, w, rsp, Alu.mult)
            _tt(nc, spn, sp, rsp, Alu.add)
            den, rden, cor = ep("den"), ep("rden"), ep("cor")
            nc.vector.tensor_scalar(
                out=den,
                in0=spn,
                scalar1=0.5,
                scalar2=(N - 1) * EPS,
                op0=Alu.mult,
                op1=Alu.add,
            )
            nc.vector.reciprocal(out=rden, in_=den)
            nc.vector.scalar_tensor_tensor(
                out=cor,
                in0=num,
                scalar=float(N) * EPS * EPS,
                in1=rden,
                op0=Alu.add,
                op1=Alu.mult,
            )
            c2, cor3 = ep("c2"), ep("cor3")
            _tt(nc, c2, cor, cor, Alu.mult)
            _tt(nc, cor3, c2, cor, Alu.mult)

            aa, lg, tl1 = ep("aa"), ep("lg"), ep("tl1")
            ln_bias = epi_pool.tile([128, 1], F32, tag="ln_bias", name="ln_bias")
            nc.vector.memset(ln_bias, 1.0 + EPS)
            nc.scalar.activation(out=aa, in_=cor3, func=Act.Abs)
            nc.scalar.activation(out=lg, in_=cor3, func=Act.Ln, bias=ln_bias)
            nc.vector.tensor_scalar(
                out=tl1,
                in0=lg,
                scalar1=-1.0,
                scalar2=float(np.log(2.0)),
                op0=Alu.mult,
                op1=Alu.add,
            )
            r1, tl2 = ep("r1"), ep("tl2")
            nc.vector.reciprocal(out=r1, in_=se1a)
            nc.vector.scalar_tensor_tensor(
                out=tl2, in0=ta[:, 0, :], scalar=1.0 / N, in1=r1, op0=Alu.mult, op1=Alu.mult
            )
            dd, pp, loss = ep("dd"), ep("pp"), ep("loss")
            _tt(nc, dd, tl1, tl2, Alu.subtract)
            _tt(nc, pp, aa, dd, Alu.mult)
            _tt(nc, loss, tl2, pp, Alu.add)

            part = epi_pool.tile([128, 1], F32, tag="part", name="part")
            nc.vector.reduce_sum(out=part, in_=loss, axis=mybir.AxisListType.X)
            nc.sync.dma_start(out=y[:, :], in_=part)


def _build_program():
    nc = bass.Bass()
    d1 = nc.dram_tensor("d1", [R, N], F32, kind="ExternalInput")
    d2 = nc.dram_tensor("d2", [R, N], F32, kind="ExternalInput")
    y = nc.dram_tensor("y", [128, 1], F32, kind="ExternalOutput")

    with tile.TileContext(nc) as tc:
        with (
            tc.tile_pool(name="data", bufs=2) as data_pool,
            tc.tile_pool(name="expp", bufs=3) as exp_pool,
            tc.tile_pool(name="gp", bufs=3) as g_pool,
            tc.tile_pool(name="fold", bufs=3) as fold_pool,
            tc.tile_pool(name="small", bufs=3) as small_pool,
            tc.tile_pool(name="stats", bufs=1) as stats_pool,
            tc.tile_pool(name="epi", bufs=1) as epi_pool,
        ):
            build_body(
                nc, tc, d1, d2, y,
                data_pool, exp_pool, g_pool, fold_pool,
                small_pool, stats_pool, epi_pool,
            )

    split_waits(nc)
    return nc


_NC = None
_RUNNER = None


def _get_nc():
    global _NC
    if _NC is None:
        _NC = _build_program()
    return _NC


def _get_runner():
    """Compile the 8-core pjrt executable once and reuse across calls."""
    global _RUNNER
    if _RUNNER is not None:
        return _RUNNER
    import jax
    from jax.sharding import Mesh, PartitionSpec
    from jax.experimental.shard_map import shard_map
    from concourse.bass2jax import (
        _bass_exec_p,
        install_neuronx_cc_hook,
        partition_id_tensor,
    )

    install_neuronx_cc_hook()
    nc = _get_nc()
    partition_name = nc.partition_id_tensor.name if nc.partition_id_tensor else None
    in_names, out_names, out_avals, zero_outs = [], [], [], []
    for alloc in nc.m.functions[0].allocations:
        if not isinstance(alloc, mybir.MemoryLocationSet):
            continue
        name = alloc.memorylocations[0].name
        if alloc.kind == "ExternalInput":
            if name != partition_name:
                in_names.append(name)
        elif alloc.kind == "ExternalOutput":
            out_names.append(name)
            shape = tuple(alloc.tensor_shape)
            dtype = mybir.dt.np(alloc.dtype)
            out_avals.append(jax.core.ShapedArray(shape, dtype))
            zero_outs.append(np.zeros(shape, dtype))
    n_params = len(in_names)
    all_in_names = list(in_names) + out_names
    if partition_name is not None:
        all_in_names.append(partition_name)

    def _body(*args):
        operands = list(args)
        if partition_name is not None:
            operands.append(partition_id_tensor())
        outs = _bass_exec_p.bind(
            *operands,
            out_avals=tuple(out_avals),
            in_names=tuple(all_in_names),
            out_names=tuple(out_names),
            lowering_input_output_aliases=(),
            sim_require_finite=True,
            sim_require_nnan=True,
            nc=nc,
        )
        return tuple(outs)

    devices = jax.devices()[:N_CORES]
    mesh = Mesh(np.asarray(devices), ("core",))
    n_outs = len(out_names)
    in_specs = (PartitionSpec("core"),) * (n_params + n_outs)
    out_specs = (PartitionSpec("core"),) * n_outs
    sharded = jax.jit(
        shard_map(
            _body, mesh=mesh, in_specs=in_specs, out_specs=out_specs,
            check_rep=False,
        ),
        keep_unused=True,
    )
    zero_cat = [
        np.zeros((N_CORES * z.shape[0], *z.shape[1:]), z.dtype) for z in zero_outs
    ]

    def run(d1, d2):
        ins = {"d1": d1, "d2": d2}
        out = sharded(*(ins[nm] for nm in in_names), *zero_cat)
        y = np.asarray(out[out_names.index("y")])
        return y

    _RUNNER = run
    return _RUNNER


def kernel(distribution1, distribution2):
    d1 = np.ascontiguousarray(np.asarray(distribution1, dtype=np.float32))
    d2 = np.ascontiguousarray(np.asarray(distribution2, dtype=np.float32))
    assert d1.shape == (B, N) and d2.shape == (B, N)
    y = _get_runner()(d1, d2)  # [N_CORES*128, 1] partial sums
    return np.asarray([np.sum(y.astype(np.float64))], dtype=np.float32)

